# revision 16
# baseline (speedup 1.0000x reference)
"""Fused GroupNorm + 4-head (HD=128) attention block + 1x1-conv projection
with residual, for x[16, 512, 32, 32], distributed data-parallel over batch
across 8 TRN2 NeuronCores (2 batch items per core, no collectives).

Math (per batch item, C=512, NH=4, HD=128, HW=1024, G=32 groups of 16 ch):
  hn = GroupNorm(x) * gamma + beta
  q = Wq hn + bq ; k = Wk hn + bk ; v = Wv hn + bv     (1x1 convs == GEMMs)
  per head h (= contiguous 128-channel slice):
    sT[pk, pq] = k_h^T . q_h   (contract over d=128)
    eT = exp(scale * sT)                               (no max-subtraction;
                                                        logits are O(1))
    out2[pq, (d, r)] = eT^T @ [v_h^T | 1]              (r = softmax denom;
                                                        the ones column comes
                                                        from the broadcast
                                                        bias tile)
    aout_h = (out2[:, :128] / r)^T                     (PE transpose)
  out = Wp aout + bp + x

All matmuls in bf16 (PSUM f32 accumulate). Emission order software-pipelines
the two batch items so ACT (softmax exp) and PE (GEMMs) overlap.
"""

import numpy as np
import ml_dtypes
from contextlib import ExitStack

B = 16
C = 512
HW = 1024
NH = 4
HD = 128
NCORES = 8
B_LOC = B // NCORES  # 2
NT = C // 128  # 4 partition tiles of channels
G = 32
GSZ = C // G  # 16 channels per group
GPT = 128 // GSZ  # 8 groups per partition-tile
EPS = 1e-5
SCALE = float(HD) ** -0.5

# vT-extended layout: per half (2 heads), each head occupies 144 cols:
# 128 v-cols, 1 ones-col (denominator), 15 pad (16B-aligned for fp8
# DoubleRow access patterns).
HSTRIDE = 144
HALF_W = 2 * HSTRIDE  # 288

_CACHE = {}


def _build_nc():
    import concourse.bass as bass
    import concourse.tile as tile
    from concourse import bacc, mybir

    f32 = mybir.dt.float32
    bf16 = mybir.dt.bfloat16
    fp8 = mybir.dt.float8e4
    DR = mybir.MatmulPerfMode.DoubleRow

    nc = bacc.Bacc()

    xp = nc.declare_dram_parameter("x", [B_LOC, C, HW], bf16, isOutput=False)
    wqT = nc.declare_dram_parameter("wqT", [C, C], bf16, isOutput=False)
    wkT = nc.declare_dram_parameter("wkT", [C, C], bf16, isOutput=False)
    wvT = nc.declare_dram_parameter("wvT", [C, 2, HALF_W], bf16, isOutput=False)
    wpT = nc.declare_dram_parameter("wpT", [C, C], bf16, isOutput=False)
    bq2 = nc.declare_dram_parameter("bq2", [128, NT], f32, isOutput=False)
    bk2 = nc.declare_dram_parameter("bk2", [128, NT], f32, isOutput=False)
    bvb = nc.declare_dram_parameter("bvb", [128, 2, HALF_W], bf16, isOutput=False)
    bp2 = nc.declare_dram_parameter("bp2", [128, NT], f32, isOutput=False)
    gam2 = nc.declare_dram_parameter("gam2", [128, NT], f32, isOutput=False)
    bet2 = nc.declare_dram_parameter("bet2", [128, NT], f32, isOutput=False)
    selS = nc.declare_dram_parameter("selS", [128, GPT], f32, isOutput=False)
    selST = nc.declare_dram_parameter("selST", [GPT, 128], f32, isOutput=False)
    iden = nc.declare_dram_parameter("iden", [128, 128], bf16, isOutput=False)
    outp = nc.declare_dram_parameter("out", [B_LOC, C, HW], f32, isOutput=True)

    Exp = mybir.ActivationFunctionType.Exp
    Ln = mybir.ActivationFunctionType.Ln
    Ident = mybir.ActivationFunctionType.Identity
    ADD = mybir.AluOpType.add
    MUL = mybir.AluOpType.mult
    DIV = mybir.AluOpType.divide
    POW = mybir.AluOpType.pow

    with tile.TileContext(nc) as tc, ExitStack() as ctx:
        wpool = ctx.enter_context(tc.tile_pool(name="wpool", bufs=1))
        px = ctx.enter_context(tc.tile_pool(name="px", bufs=2 * NT))
        phn = ctx.enter_context(tc.tile_pool(name="phn", bufs=2 * NT))
        pq = ctx.enter_context(tc.tile_pool(name="pq", bufs=2 * NT))
        pk = ctx.enter_context(tc.tile_pool(name="pk", bufs=2 * NT))
        pv = ctx.enter_context(tc.tile_pool(name="pv", bufs=16))
        pe = ctx.enter_context(tc.tile_pool(name="pe", bufs=3))
        pa = ctx.enter_context(tc.tile_pool(name="pa", bufs=2 * NH))
        po = ctx.enter_context(tc.tile_pool(name="po", bufs=4))
        psc = ctx.enter_context(tc.tile_pool(name="psc", bufs=3))
        prc = ctx.enter_context(tc.tile_pool(name="prc", bufs=4))
        ps = ctx.enter_context(tc.tile_pool(name="ps", bufs=2, space="PSUM"))

        # --- x for batch 0 first (it heads the critical path to hn/QKV),
        # then the small GN constants, then weights, then x for batch 1 ---
        xt = [[None] * NT for _ in range(B_LOC)]
        hn = [[None] * NT for _ in range(B_LOC)]
        for t in range(NT):
            xt[0][t] = px.tile([128, HW], bf16, name=f"x_0_{t}", tag="x")
            nc.gpsimd.dma_start(out=xt[0][t][:], in_=xp[0, 128 * t : 128 * (t + 1), :])

        s_bq = wpool.tile([128, NT], f32, name="s_bq")
        s_bk = wpool.tile([128, NT], f32, name="s_bk")
        s_bp = wpool.tile([128, NT], f32, name="s_bp")
        s_gam = wpool.tile([128, NT], f32, name="s_gam")
        s_bet = wpool.tile([128, NT], f32, name="s_bet")
        s_bvb = wpool.tile([128, 2, HALF_W], bf16, name="s_bvb")
        s_selS = wpool.tile([128, GPT], f32, name="s_selS")
        s_selST = wpool.tile([GPT, 128], f32, name="s_selST")
        s_iden = wpool.tile([128, 128], bf16, name="s_iden")
        s_m2 = wpool.tile([128, 1], f32, name="s_m2")
        nc.vector.memset(s_m2[:], -2.0)
        s_eps = wpool.tile([128, 1], f32, name="s_eps")
        nc.vector.memset(s_eps[:], EPS)
        nc.gpsimd.dma_start(out=s_selS[:], in_=selS[:])
        nc.gpsimd.dma_start(out=s_selST[:], in_=selST[:])
        nc.gpsimd.dma_start(out=s_gam[:], in_=gam2[:])
        nc.gpsimd.dma_start(out=s_bet[:], in_=bet2[:])
        nc.gpsimd.dma_start(out=s_bq[:], in_=bq2[:])
        nc.gpsimd.dma_start(out=s_bk[:], in_=bk2[:])
        nc.gpsimd.dma_start(out=s_bp[:], in_=bp2[:])
        nc.gpsimd.dma_start(out=s_bvb[:], in_=bvb[:])
        nc.gpsimd.dma_start(out=s_iden[:], in_=iden[:])

        w_q = wpool.tile([128, NT, C], bf16, name="w_q")
        w_k = wpool.tile([128, NT, C], bf16, name="w_k")
        w_p = wpool.tile([128, NT, C], bf16, name="w_p")
        w_v = wpool.tile([128, NT, 2, HALF_W], bf16, name="w_v")
        for t in range(NT):
            nc.gpsimd.dma_start(out=w_q[:, t, :], in_=wqT[128 * t : 128 * (t + 1), :])
            nc.gpsimd.dma_start(out=w_k[:, t, :], in_=wkT[128 * t : 128 * (t + 1), :])
        for t in range(NT):
            xt[1][t] = px.tile([128, HW], bf16, name=f"x_1_{t}", tag="x")
            nc.gpsimd.dma_start(out=xt[1][t][:], in_=xp[1, 128 * t : 128 * (t + 1), :])
        for t in range(NT):
            nc.gpsimd.dma_start(out=w_v[:, t, :, :], in_=wvT[128 * t : 128 * (t + 1), :, :])
            nc.gpsimd.dma_start(out=w_p[:, t, :], in_=wpT[128 * t : 128 * (t + 1), :])

        # benchmark mode: execute the whole body R times in a hardware loop
        # so per-iteration time can be measured through dispatch noise
        import os as _os0

        R_BENCH = int(_os0.environ.get("BENCHR", "1"))
        if R_BENCH > 1:
            ctx.enter_context(tc.For_i(0, R_BENCH, 1))
        # Python-level unroll for TimelineSim steady-state measurement
        # (TimelineSim cannot follow the For_i register branch).
        UNROLL = int(_os0.environ.get("UNROLL", "1"))
        IT = [0]

        # ---------------- GroupNorm ----------------
        def _gn(b):
            # per-channel stats -> g_in[:, 3t+(0,1,2)] = mean, var, mean^2
            g_in = psc.tile([128, 3 * NT], f32, name=f"g_in_{b}i{IT[0]}", tag="g_in")
            for t in range(NT):
                st6 = psc.tile([128, 2, 6], f32, name=f"st6_{b}_{t}i{IT[0]}", tag="st6")
                nc.vector.bn_stats(out=st6[:, 0, :], in_=xt[b][t][:, 0:512])
                nc.vector.bn_stats(out=st6[:, 1, :], in_=xt[b][t][:, 512:1024])
                nc.vector.bn_aggr(
                    out=g_in[:, 3 * t : 3 * t + 2], in_=st6[:, :, :]
                )
            nc.vector.tensor_mul(
                out=g_in[:, 2::3], in0=g_in[:, 0::3], in1=g_in[:, 0::3]
            )

            # aggregate over the 16 channels of each group (sum across
            # partitions via selector matmul; groups are 16 consecutive
            # channels so group j-of-tile-t = partitions 16j..16j+15).
            g_ps = ps.tile([GPT, 3 * NT], f32, name=f"g_ps_{b}i{IT[0]}", tag="mm")
            nc.tensor.matmul(
                out=g_ps[:], lhsT=s_selS[:], rhs=g_in[:], start=True, stop=True
            )
            gs = psc.tile([GPT, 3 * NT], f32, name=f"gs_{b}i{IT[0]}", tag="gs")
            nc.vector.tensor_scalar_mul(out=gs[:], in0=g_ps[:], scalar1=1.0 / GSZ)
            # group var = E[var] + E[mean^2] - mean_g^2 ; rstd = (var+eps)^-1/2
            vg = psc.tile([GPT, NT], f32, name=f"vg_{b}i{IT[0]}", tag="vg")
            mg2 = psc.tile([GPT, NT], f32, name=f"mg2_{b}i{IT[0]}", tag="mg2")
            nc.vector.tensor_mul(out=mg2[:], in0=gs[:, 0::3], in1=gs[:, 0::3])
            nc.vector.tensor_add(out=vg[:], in0=gs[:, 1::3], in1=gs[:, 2::3])
            nc.vector.tensor_sub(out=vg[:], in0=vg[:], in1=mg2[:])
            # rstd = rsqrt(var+eps) via Newton iteration on DVE (avoids
            # pulling a second activation table beside the softmax Exp).
            # var is tightly concentrated near 1.0 (x ~ N(0,1), 16k-sample
            # groups), so y0=1: y1 = 1.5-0.5v, then y <- y*(1.5-0.5*v*y^2).
            nc.vector.tensor_scalar_add(out=vg[:], in0=vg[:], scalar1=EPS)
            rstd = psc.tile([GPT, NT], f32, name=f"rstd_{b}i{IT[0]}", tag="rstd")
            nc.vector.tensor_scalar(
                out=rstd[:], in0=vg[:], scalar1=-0.5, scalar2=1.5,
                op0=MUL, op1=ADD,
            )
            nwt = psc.tile([GPT, NT], f32, name=f"nwt_{b}i{IT[0]}", tag="nwt")
            for _ in range(2):
                nc.vector.tensor_mul(out=nwt[:], in0=rstd[:], in1=rstd[:])
                nc.vector.tensor_mul(out=nwt[:], in0=nwt[:], in1=vg[:])
                nc.vector.tensor_scalar(
                    out=nwt[:], in0=nwt[:], scalar1=-0.5, scalar2=1.5,
                    op0=MUL, op1=ADD,
                )
                nc.vector.tensor_mul(out=rstd[:], in0=rstd[:], in1=nwt[:])

            # broadcast group stats back to channels: bc[:, 3t]=mean_g(ch),
            # bc[:, 12+t]=rstd(ch)
            bc_ps = ps.tile([128, 4 * NT], f32, name=f"bc_ps_{b}i{IT[0]}", tag="mm")
            nc.tensor.matmul(
                out=bc_ps[:, 0 : 3 * NT],
                lhsT=s_selST[:],
                rhs=gs[:],
                start=True,
                stop=True,
            )
            nc.tensor.matmul(
                out=bc_ps[:, 3 * NT : 4 * NT],
                lhsT=s_selST[:],
                rhs=rstd[:],
                start=True,
                stop=True,
            )
            # per-channel affine: a = gamma*rstd ; bb = beta - mean*a
            av = psc.tile([128, NT], f32, name=f"av_{b}i{IT[0]}", tag="av")
            bb = psc.tile([128, NT], f32, name=f"bb_{b}i{IT[0]}", tag="bb")
            nc.vector.tensor_mul(
                out=av[:], in0=s_gam[:], in1=bc_ps[:, 3 * NT : 4 * NT]
            )
            nc.vector.tensor_mul(out=bb[:], in0=bc_ps[:, 0 : 3 * NT : 3], in1=av[:])
            nc.vector.tensor_sub(out=bb[:], in0=s_bet[:], in1=bb[:])
            for t in range(NT):
                hn[b][t] = phn.tile([128, HW], bf16, name=f"hn_{b}_{t}i{IT[0]}", tag="hn")
                eng = nc.vector if t < 2 else nc.gpsimd
                eng.tensor_scalar(
                    out=hn[b][t][:],
                    in0=xt[b][t][:],
                    scalar1=av[:, t : t + 1],
                    scalar2=bb[:, t : t + 1],
                    op0=MUL,
                    op1=ADD,
                )

        # ---------------- QKV ----------------
        q_sb = [[None] * NT for _ in range(B_LOC)]
        k_sb = [[None] * NT for _ in range(B_LOC)]
        vT = [[None] * 8 for _ in range(B_LOC)]

        def _qk_unit(b, ot, which, half):
            # q[o,p] = sum_c wqT[c,o] hn[c,p]  (+bias on evac)
            if which == "q" and half == 0:
                q_sb[b][ot] = pq.tile([128, HW], bf16, name=f"q_{b}_{ot}i{IT[0]}", tag="q")
                k_sb[b][ot] = pk.tile([128, HW], bf16, name=f"k_{b}_{ot}i{IT[0]}", tag="k")
            wsb, bias_sb, dst = (
                (w_q, s_bq, q_sb[b][ot]) if which == "q"
                else (w_k, s_bk, k_sb[b][ot])
            )
            mm = ps.tile(
                [128, 512], f32, name=f"mm_{which}_{b}_{ot}_{half}i{IT[0]}",
                tag="mm",
            )
            for ct in range(NT):
                nc.tensor.matmul(
                    out=mm[:],
                    lhsT=wsb[:, ct, 128 * ot : 128 * (ot + 1)],
                    rhs=hn[b][ct][:, 512 * half : 512 * (half + 1)],
                    start=(ct == 0),
                    stop=(ct == NT - 1),
                )
            if b == 0:
                nc.scalar.activation(
                    out=dst[:, 512 * half : 512 * (half + 1)],
                    in_=mm[:],
                    func=Ident,
                    bias=bias_sb[:, ot : ot + 1],
                    scale=1.0,
                )
            else:
                nc.vector.tensor_scalar_add(
                    out=dst[:, 512 * half : 512 * (half + 1)],
                    in0=mm[:],
                    scalar1=bias_sb[:, ot : ot + 1],
                )

        def _v_unit(b, cch, half2):
            # vT[pk, o] = sum_c hn[c,pk] wvT[c,o]; bias added via the
            # precomputed broadcast tile (also provides the ones column
            # for the softmax denominator). Output is written as fp8
            # k-pair slabs [128, 2, 2, HALF_W] (dim1 = k-tile pair member)
            # so the AV matmuls can run in DoubleRow mode.
            cp, j = divmod(cch, 2)
            if j == 0 and half2 == 0:
                vT[b][cp] = pv.tile(
                    [128, 2, 2, HALF_W], fp8,
                    name=f"vT_{b}_{cp}i{IT[0]}", tag="vT"
                )
            mmv = ps.tile(
                [128, HALF_W], f32, name=f"mmv_{b}_{cch}_{half2}i{IT[0]}", tag="mm"
            )
            for ct in range(NT):
                nc.tensor.matmul(
                    out=mmv[:],
                    lhsT=hn[b][ct][:, 128 * cch : 128 * (cch + 1)],
                    rhs=w_v[:, ct, half2, :],
                    start=(ct == 0),
                    stop=(ct == NT - 1),
                )
            nc.vector.tensor_tensor(
                out=vT[b][cp][:, j, half2, :],
                in0=mmv[:],
                in1=s_bvb[:, half2, :],
                op=ADD,
            )

        # ---------------- attention ----------------
        eT = {}  # (b, h) -> list of 8 tiles
        aout = [[None] * NH for _ in range(B_LOC)]

        def _scores_unit(b, h, cch):
            # eT = exp(scale*s - 2); the constant shift cancels in softmax
            # and keeps exp outputs < 240 (fp8e4 saturates to Inf above).
            cp, j = divmod(cch, 2)
            if cch == 0:
                eT[(b, h)] = [None] * 4
            sps = ps.tile(
                [128, HW], f32, name=f"sps_{b}_{h}_{cch}i{IT[0]}", tag="sc"
            )
            for half in range(2):
                nc.tensor.matmul(
                    out=sps[:, 512 * half : 512 * (half + 1)],
                    lhsT=k_sb[b][h][:, 128 * cch : 128 * (cch + 1)],
                    rhs=q_sb[b][h][:, 512 * half : 512 * (half + 1)],
                    start=True,
                    stop=True,
                )
            if j == 0:
                eT[(b, h)][cp] = pe.tile(
                    [128, 2, HW], fp8,
                    name=f"eT_{b}_{h}_{cp}i{IT[0]}", tag=f"eT{cp}"
                )
            nc.scalar.activation(
                out=eT[(b, h)][cp][:, j, :], in_=sps[:], func=Exp,
                bias=s_m2[:], scale=SCALE
            )

        o2n_cur = {}

        def _av_unit(b, h, m):
            if m == 0:
                # aout stored as [128, 8, 128] m-blocks (contiguous == HW)
                aout[b][h] = pa.tile(
                    [128, 8, 128], bf16, name=f"aout_{b}_{h}i{IT[0]}", tag="aout"
                )
                o2n_cur[(b, h)] = prc.tile(
                    [128, 8, 128], bf16, name=f"o2n_{b}_{h}i{IT[0]}", tag="o2n"
                )
            half2, hh = divmod(h, 2)
            voff = HSTRIDE * hh
            ets = eT[(b, h)]
            o2n = o2n_cur[(b, h)]
            ops = ps.tile(
                [128, HD + 1], f32, name=f"ops_{b}_{h}_{m}i{IT[0]}", tag="av"
            )
            for cp in range(4):
                nc.tensor.matmul(
                    out=ops[:],
                    lhsT=ets[cp][:, :, 128 * m : 128 * (m + 1)],
                    rhs=vT[b][cp][:, :, half2, voff : voff + HD + 1],
                    start=(cp == 0),
                    stop=(cp == 3),
                    perf_mode=DR,
                )
            rcp = prc.tile([128, 1], f32, name=f"rcp_{b}_{h}_{m}i{IT[0]}", tag="rcp")
            nc.vector.reciprocal(out=rcp[:], in_=ops[:, HD : HD + 1])
            nc.vector.tensor_scalar_mul(
                out=o2n[:, m, :], in0=ops[:, 0:HD], scalar1=rcp[:]
            )
            if m == 3 or m == 7:
                # batched xbar transpose per half-head: aout[p, m, c] =
                # o2n[c, m, p] (block-transpose of each 128x128 m-chunk)
                mlo = m - 3
                nc.sync.dma_start_transpose(
                    out=aout[b][h][:, mlo : m + 1, :],
                    in_=o2n[:, mlo : m + 1, :],
                )

        # ---------------- projection + bias + residual ----------------
        o_cur = {}

        def _proj_unit(b, ot, half):
            if half == 0:
                o_cur[(b, ot)] = po.tile(
                    [128, HW], f32, name=f"o_{b}_{ot}i{IT[0]}", tag="o"
                )
            o_sb = o_cur[(b, ot)]
            mmp = ps.tile(
                [128, 512], f32, name=f"mmp_{b}_{ot}_{half}i{IT[0]}", tag="mm"
            )
            for ct in range(NT):
                nc.tensor.matmul(
                    out=mmp[:],
                    lhsT=w_p[:, ct, 128 * ot : 128 * (ot + 1)],
                    rhs=aout[b][ct][:, 4 * half : 4 * (half + 1), :],
                    start=(ct == 0),
                    stop=(ct == NT - 1),
                )
            nc.vector.scalar_tensor_tensor(
                out=o_sb[:, 512 * half : 512 * (half + 1)],
                in0=mmp[:],
                scalar=s_bp[:, ot : ot + 1],
                in1=xt[b][ot][:, 512 * half : 512 * (half + 1)],
                op0=ADD,
                op1=ADD,
            )
            if half == 1:
                nc.gpsimd.dma_start(
                    out=outp[b, 128 * ot : 128 * (ot + 1), :], in_=o_sb[:]
                )

        # ---------------- emission order: software pipeline ----------------
        # Unit-granular interleave: the exp stream (ACT) is paced by one
        # scores-unit per slot; each slot also carries AV units of an
        # earlier head plus GEMM "filler" units so the PE queue head is
        # never blocked on a single dependency chain.
        from collections import deque

        for _it in range(UNROLL):
            IT[0] = _it
            _gn(0)
            _gn(1)

            fill = deque()
            for b2 in range(B_LOC):
                for ot in range(NT):
                    for which in ("q", "k"):
                        for half in range(2):
                            fill.append((_qk_unit, (b2, ot, which, half)))
                if b2 == 0:
                    for cch in range(8):
                        for half2 in range(2):
                            fill.append((_v_unit, (0, cch, half2)))
            for cch in range(8):
                for half2 in range(2):
                    fill.append((_v_unit, (1, cch, half2)))

            def _fill(n):
                for _ in range(n):
                    if fill:
                        f, a = fill.popleft()
                        f(*a)

            # prologue: hn(0) -> q/k for head 0
            _fill(4)
            heads = [(0, 0), (0, 1), (0, 2), (0, 3), (1, 0), (1, 1), (1, 2), (1, 3)]
            # av streams emitted during each head slot (lagging so vT /
            # eT producers are comfortably ahead)
            av_sched = {2: [(0, 0), (0, 1)], 3: [(0, 2)], 4: [(0, 3)],
                        6: [(1, 0), (1, 1)], 7: [(1, 2)]}
            for hi, (b2, h) in enumerate(heads):
                avs = av_sched.get(hi, [])
                for cch in range(8):
                    _scores_unit(b2, h, cch)
                    for abh in avs:
                        _av_unit(abh[0], abh[1], cch)
                    _fill(2 if hi < 2 else 1)
                if hi == 4:
                    for ot in range(NT):
                        for half in range(2):
                            fill.append((_proj_unit, (0, ot, half)))
            # tail: last head's AV, leftover fillers, final projection
            for m in range(8):
                _av_unit(1, 3, m)
                _fill(1)
            _fill(64)
            for ot in range(NT):
                for half in range(2):
                    _proj_unit(1, ot, half)

    nc.compile()
    return nc


def _get_nc():
    if "nc" not in _CACHE:
        _CACHE["nc"] = _build_nc()
    return _CACHE["nc"]


def _prep_shared(wq, bq, wk, bk, wv, bv, wp, bp, gamma, beta):
    bf16 = ml_dtypes.bfloat16
    wqT = np.ascontiguousarray(wq.T).astype(bf16)
    wkT = np.ascontiguousarray(wk.T).astype(bf16)
    wpT = np.ascontiguousarray(wp.T).astype(bf16)
    wvTf = np.ascontiguousarray(wv.T)  # [c, o]
    wvT = np.zeros((C, 2, HALF_W), dtype=bf16)
    bvb = np.zeros((1, 2, HALF_W), dtype=np.float32)
    for h in range(NH):
        half2, hh = divmod(h, 2)
        off = HSTRIDE * hh
        wvT[:, half2, off : off + HD] = wvTf[:, HD * h : HD * (h + 1)].astype(bf16)
        bvb[0, half2, off : off + HD] = bv[HD * h : HD * (h + 1)]
        bvb[0, half2, off + HD] = 1.0  # ones column -> softmax denominator
    bvb = np.broadcast_to(bvb, (128, 2, HALF_W)).astype(bf16)
    as_col = lambda v: np.ascontiguousarray(v.reshape(NT, 128).T).astype(np.float32)
    selS = np.zeros((128, GPT), np.float32)
    for p in range(128):
        selS[p, p // GSZ] = 1.0
    selST = np.ascontiguousarray(selS.T)
    iden = np.eye(128, dtype=bf16)
    return {
        "wqT": wqT,
        "wkT": wkT,
        "wvT": wvT,
        "wpT": wpT,
        "bq2": as_col(bq),
        "bk2": as_col(bk),
        "bvb": np.ascontiguousarray(bvb),
        "bp2": as_col(bp),
        "gam2": as_col(gamma),
        "bet2": as_col(beta),
        "selS": selS,
        "selST": selST,
        "iden": iden,
    }


def kernel(x, gamma, beta, wq, bq, wk, bk, wv, bv, wp, bp, _trace=False):
    from concourse.bass_utils import run_bass_kernel_spmd

    x = np.asarray(x, dtype=np.float32)
    shared = _prep_shared(
        np.asarray(wq, np.float32),
        np.asarray(bq, np.float32),
        np.asarray(wk, np.float32),
        np.asarray(bk, np.float32),
        np.asarray(wv, np.float32),
        np.asarray(bv, np.float32),
        np.asarray(wp, np.float32),
        np.asarray(bp, np.float32),
        np.asarray(gamma, np.float32),
        np.asarray(beta, np.float32),
    )
    xr = x.reshape(B, C, HW).astype(ml_dtypes.bfloat16)
    in_maps = []
    for i in range(NCORES):
        m = dict(shared)
        m["x"] = np.ascontiguousarray(xr[B_LOC * i : B_LOC * (i + 1)])
        in_maps.append(m)

    nc = _get_nc()
    res = run_bass_kernel_spmd(
        nc, in_maps, core_ids=list(range(NCORES)), trace=_trace
    )
    out = np.concatenate([res.results[i]["out"] for i in range(NCORES)], axis=0)
    out = out.reshape(B, C, 32, 32).astype(np.float32)
    if _trace:
        _CACHE["last_exec_time_ns"] = res.exec_time_ns
        _CACHE["last_results"] = res
    return out


# revision 19
# speedup vs baseline: 1.0935x; 1.0935x over previous
"""Fused GroupNorm + 4-head (HD=128) attention block + 1x1-conv projection
with residual, for x[16, 512, 32, 32], distributed data-parallel over batch
across 8 TRN2 NeuronCores (2 batch items per core, no collectives).

Math (per batch item, C=512, NH=4, HD=128, HW=1024, G=32 groups of 16 ch):
  hn = GroupNorm(x) * gamma + beta
  q = Wq hn + bq ; k = Wk hn + bk ; v = Wv hn + bv     (1x1 convs == GEMMs)
  per head h (= contiguous 128-channel slice):
    sT[pk, pq] = k_h^T . q_h   (contract over d=128)
    eT = exp(scale * sT)                               (no max-subtraction;
                                                        logits are O(1))
    out2[pq, (d, r)] = eT^T @ [v_h^T | 1]              (r = softmax denom;
                                                        the ones column comes
                                                        from the broadcast
                                                        bias tile)
    aout_h = (out2[:, :128] / r)^T                     (PE transpose)
  out = Wp aout + bp + x

All matmuls in bf16 (PSUM f32 accumulate). Emission order software-pipelines
the two batch items so ACT (softmax exp) and PE (GEMMs) overlap.
"""

import numpy as np
import ml_dtypes
from contextlib import ExitStack

B = 16
C = 512
HW = 1024
NH = 4
HD = 128
NCORES = 8
B_LOC = B // NCORES  # 2
NT = C // 128  # 4 partition tiles of channels
G = 32
GSZ = C // G  # 16 channels per group
GPT = 128 // GSZ  # 8 groups per partition-tile
EPS = 1e-5
SCALE = float(HD) ** -0.5

# vT-extended layout: per half (2 heads), each head occupies 144 cols:
# 128 v-cols, 1 ones-col (denominator), 15 pad (16B-aligned for fp8
# DoubleRow access patterns).
HSTRIDE = 144
HALF_W = 2 * HSTRIDE  # 288

_CACHE = {}


def _build_nc():
    import concourse.bass as bass
    import concourse.tile as tile
    from concourse import bacc, mybir

    f32 = mybir.dt.float32
    bf16 = mybir.dt.bfloat16
    fp8 = mybir.dt.float8e4
    DR = mybir.MatmulPerfMode.DoubleRow

    nc = bacc.Bacc()

    xp = nc.declare_dram_parameter("x", [B_LOC, C, HW], bf16, isOutput=False)
    wqT = nc.declare_dram_parameter("wqT", [C, C], bf16, isOutput=False)
    wkT = nc.declare_dram_parameter("wkT", [C, C], bf16, isOutput=False)
    wvT = nc.declare_dram_parameter("wvT", [C, 2, HALF_W], bf16, isOutput=False)
    wpT = nc.declare_dram_parameter("wpT", [C, C], bf16, isOutput=False)
    bq2 = nc.declare_dram_parameter("bq2", [128, NT], f32, isOutput=False)
    bk2 = nc.declare_dram_parameter("bk2", [128, NT], f32, isOutput=False)
    bvb = nc.declare_dram_parameter("bvb", [128, 2, HALF_W], bf16, isOutput=False)
    bp2 = nc.declare_dram_parameter("bp2", [128, NT], f32, isOutput=False)
    gam2 = nc.declare_dram_parameter("gam2", [128, NT], f32, isOutput=False)
    bet2 = nc.declare_dram_parameter("bet2", [128, NT], f32, isOutput=False)
    selS = nc.declare_dram_parameter("selS", [128, GPT], f32, isOutput=False)
    selST = nc.declare_dram_parameter("selST", [GPT, 128], f32, isOutput=False)
    iden = nc.declare_dram_parameter("iden", [128, 128], bf16, isOutput=False)
    outp = nc.declare_dram_parameter("out", [B_LOC, C, HW], f32, isOutput=True)

    Exp = mybir.ActivationFunctionType.Exp
    Ln = mybir.ActivationFunctionType.Ln
    ADD = mybir.AluOpType.add
    MUL = mybir.AluOpType.mult
    DIV = mybir.AluOpType.divide
    POW = mybir.AluOpType.pow

    with tile.TileContext(nc) as tc, ExitStack() as ctx:
        wpool = ctx.enter_context(tc.tile_pool(name="wpool", bufs=1))
        px = ctx.enter_context(tc.tile_pool(name="px", bufs=2 * NT))
        phn = ctx.enter_context(tc.tile_pool(name="phn", bufs=2 * NT))
        pq = ctx.enter_context(tc.tile_pool(name="pq", bufs=2 * NT))
        pk = ctx.enter_context(tc.tile_pool(name="pk", bufs=2 * NT))
        pv = ctx.enter_context(tc.tile_pool(name="pv", bufs=16))
        pe = ctx.enter_context(tc.tile_pool(name="pe", bufs=3))
        pa = ctx.enter_context(tc.tile_pool(name="pa", bufs=2 * NH))
        po = ctx.enter_context(tc.tile_pool(name="po", bufs=4))
        psc = ctx.enter_context(tc.tile_pool(name="psc", bufs=3))
        prc = ctx.enter_context(tc.tile_pool(name="prc", bufs=4))
        ps = ctx.enter_context(tc.tile_pool(name="ps", bufs=2, space="PSUM"))

        # --- x for batch 0 first (it heads the critical path to hn/QKV),
        # then the small GN constants, then weights, then x for batch 1 ---
        xt = [[None] * NT for _ in range(B_LOC)]
        hn = [[None] * NT for _ in range(B_LOC)]
        for t in range(NT):
            xt[0][t] = px.tile([128, HW], bf16, name=f"x_0_{t}", tag="x")
            nc.gpsimd.dma_start(out=xt[0][t][:], in_=xp[0, 128 * t : 128 * (t + 1), :])

        s_bq = wpool.tile([128, NT], f32, name="s_bq")
        s_bk = wpool.tile([128, NT], f32, name="s_bk")
        s_bp = wpool.tile([128, NT], f32, name="s_bp")
        s_gam = wpool.tile([128, NT], f32, name="s_gam")
        s_bet = wpool.tile([128, NT], f32, name="s_bet")
        s_bvb = wpool.tile([128, 2, HALF_W], bf16, name="s_bvb")
        s_selS = wpool.tile([128, GPT], f32, name="s_selS")
        s_selST = wpool.tile([GPT, 128], f32, name="s_selST")
        s_iden = wpool.tile([128, 128], bf16, name="s_iden")
        s_m2 = wpool.tile([128, 1], f32, name="s_m2")
        nc.vector.memset(s_m2[:], -2.0)
        s_eps = wpool.tile([128, 1], f32, name="s_eps")
        nc.vector.memset(s_eps[:], EPS)
        nc.gpsimd.dma_start(out=s_selS[:], in_=selS[:])
        nc.gpsimd.dma_start(out=s_selST[:], in_=selST[:])
        nc.gpsimd.dma_start(out=s_gam[:], in_=gam2[:])
        nc.gpsimd.dma_start(out=s_bet[:], in_=bet2[:])
        nc.gpsimd.dma_start(out=s_bq[:], in_=bq2[:])
        nc.gpsimd.dma_start(out=s_bk[:], in_=bk2[:])
        nc.gpsimd.dma_start(out=s_bp[:], in_=bp2[:])
        nc.gpsimd.dma_start(out=s_bvb[:], in_=bvb[:])
        nc.gpsimd.dma_start(out=s_iden[:], in_=iden[:])

        w_q = wpool.tile([128, NT, C], bf16, name="w_q")
        w_k = wpool.tile([128, NT, C], bf16, name="w_k")
        w_p = wpool.tile([128, NT, C], bf16, name="w_p")
        w_v = wpool.tile([128, NT, 2, HALF_W], bf16, name="w_v")
        for t in range(NT):
            nc.gpsimd.dma_start(out=w_q[:, t, :], in_=wqT[128 * t : 128 * (t + 1), :])
            nc.gpsimd.dma_start(out=w_k[:, t, :], in_=wkT[128 * t : 128 * (t + 1), :])
        for t in range(NT):
            xt[1][t] = px.tile([128, HW], bf16, name=f"x_1_{t}", tag="x")
            nc.gpsimd.dma_start(out=xt[1][t][:], in_=xp[1, 128 * t : 128 * (t + 1), :])
        for t in range(NT):
            nc.gpsimd.dma_start(out=w_v[:, t, :, :], in_=wvT[128 * t : 128 * (t + 1), :, :])
            nc.gpsimd.dma_start(out=w_p[:, t, :], in_=wpT[128 * t : 128 * (t + 1), :])

        # benchmark mode: execute the whole body R times in a hardware loop
        # so per-iteration time can be measured through dispatch noise
        import os as _os0

        R_BENCH = int(_os0.environ.get("BENCHR", "1"))
        if R_BENCH > 1:
            ctx.enter_context(tc.For_i(0, R_BENCH, 1))
        # Python-level unroll for TimelineSim steady-state measurement
        # (TimelineSim cannot follow the For_i register branch).
        UNROLL = int(_os0.environ.get("UNROLL", "1"))
        IT = [0]

        # ---------------- GroupNorm ----------------
        def _gn(b):
            # per-channel stats -> g_in[:, 3t+(0,1,2)] = mean, var, mean^2
            g_in = psc.tile([128, 3 * NT], f32, name=f"g_in_{b}i{IT[0]}", tag="g_in")
            for t in range(NT):
                st6 = psc.tile([128, 2, 6], f32, name=f"st6_{b}_{t}i{IT[0]}", tag="st6")
                nc.vector.bn_stats(out=st6[:, 0, :], in_=xt[b][t][:, 0:512])
                nc.vector.bn_stats(out=st6[:, 1, :], in_=xt[b][t][:, 512:1024])
                nc.vector.bn_aggr(
                    out=g_in[:, 3 * t : 3 * t + 2], in_=st6[:, :, :]
                )
            nc.vector.tensor_mul(
                out=g_in[:, 2::3], in0=g_in[:, 0::3], in1=g_in[:, 0::3]
            )

            # aggregate over the 16 channels of each group (sum across
            # partitions via selector matmul; groups are 16 consecutive
            # channels so group j-of-tile-t = partitions 16j..16j+15).
            g_ps = ps.tile([GPT, 3 * NT], f32, name=f"g_ps_{b}i{IT[0]}", tag="mm")
            nc.tensor.matmul(
                out=g_ps[:], lhsT=s_selS[:], rhs=g_in[:], start=True, stop=True
            )
            gs = psc.tile([GPT, 3 * NT], f32, name=f"gs_{b}i{IT[0]}", tag="gs")
            nc.vector.tensor_scalar_mul(out=gs[:], in0=g_ps[:], scalar1=1.0 / GSZ)
            # group var = E[var] + E[mean^2] - mean_g^2 ; rstd = (var+eps)^-1/2
            vg = psc.tile([GPT, NT], f32, name=f"vg_{b}i{IT[0]}", tag="vg")
            mg2 = psc.tile([GPT, NT], f32, name=f"mg2_{b}i{IT[0]}", tag="mg2")
            nc.vector.tensor_mul(out=mg2[:], in0=gs[:, 0::3], in1=gs[:, 0::3])
            nc.vector.tensor_add(out=vg[:], in0=gs[:, 1::3], in1=gs[:, 2::3])
            nc.vector.tensor_sub(out=vg[:], in0=vg[:], in1=mg2[:])
            # rstd = exp(-0.5*ln(var+eps)); Ln+Exp share one activation
            # table with the softmax Exp (natural_log_exp_and_others)
            lnv = psc.tile([GPT, NT], f32, name=f"lnv_{b}i{IT[0]}", tag="lnv")
            nc.scalar.activation(
                out=lnv[:], in_=vg[:], func=Ln, bias=s_eps[:GPT], scale=1.0
            )
            rstd = psc.tile([GPT, NT], f32, name=f"rstd_{b}i{IT[0]}", tag="rstd")
            nc.scalar.activation(
                out=rstd[:], in_=lnv[:], func=Exp, bias=0.0, scale=-0.5
            )

            # broadcast group stats back to channels: bc[:, 3t]=mean_g(ch),
            # bc[:, 12+t]=rstd(ch)
            bc_ps = ps.tile([128, 4 * NT], f32, name=f"bc_ps_{b}i{IT[0]}", tag="mm")
            nc.tensor.matmul(
                out=bc_ps[:, 0 : 3 * NT],
                lhsT=s_selST[:],
                rhs=gs[:],
                start=True,
                stop=True,
            )
            nc.tensor.matmul(
                out=bc_ps[:, 3 * NT : 4 * NT],
                lhsT=s_selST[:],
                rhs=rstd[:],
                start=True,
                stop=True,
            )
            # per-channel affine: a = gamma*rstd ; bb = beta - mean*a
            av = psc.tile([128, NT], f32, name=f"av_{b}i{IT[0]}", tag="av")
            bb = psc.tile([128, NT], f32, name=f"bb_{b}i{IT[0]}", tag="bb")
            nc.vector.tensor_mul(
                out=av[:], in0=s_gam[:], in1=bc_ps[:, 3 * NT : 4 * NT]
            )
            nc.vector.tensor_mul(out=bb[:], in0=bc_ps[:, 0 : 3 * NT : 3], in1=av[:])
            nc.vector.tensor_sub(out=bb[:], in0=s_bet[:], in1=bb[:])
            for t in range(NT):
                hn[b][t] = phn.tile([128, HW], bf16, name=f"hn_{b}_{t}i{IT[0]}", tag="hn")
                nc.gpsimd.tensor_scalar(
                    out=hn[b][t][:],
                    in0=xt[b][t][:],
                    scalar1=av[:, t : t + 1],
                    scalar2=bb[:, t : t + 1],
                    op0=MUL,
                    op1=ADD,
                )

        # ---------------- QKV ----------------
        q_sb = [[None] * NT for _ in range(B_LOC)]
        k_sb = [[None] * NT for _ in range(B_LOC)]
        vT = [[None] * 8 for _ in range(B_LOC)]

        def _qk_unit(b, ot, which, half):
            # q[o,p] = sum_c wqT[c,o] hn[c,p]  (+bias on evac)
            if which == "q" and half == 0:
                q_sb[b][ot] = pq.tile([128, HW], bf16, name=f"q_{b}_{ot}i{IT[0]}", tag="q")
                k_sb[b][ot] = pk.tile([128, HW], bf16, name=f"k_{b}_{ot}i{IT[0]}", tag="k")
            wsb, bias_sb, dst = (
                (w_q, s_bq, q_sb[b][ot]) if which == "q"
                else (w_k, s_bk, k_sb[b][ot])
            )
            mm = ps.tile(
                [128, 512], f32, name=f"mm_{which}_{b}_{ot}_{half}i{IT[0]}",
                tag="mm",
            )
            for ct in range(NT):
                nc.tensor.matmul(
                    out=mm[:],
                    lhsT=wsb[:, ct, 128 * ot : 128 * (ot + 1)],
                    rhs=hn[b][ct][:, 512 * half : 512 * (half + 1)],
                    start=(ct == 0),
                    stop=(ct == NT - 1),
                )
            nc.vector.tensor_scalar_add(
                out=dst[:, 512 * half : 512 * (half + 1)],
                in0=mm[:],
                scalar1=bias_sb[:, ot : ot + 1],
            )

        def _v_unit(b, cch, half2):
            # vT[pk, o] = sum_c hn[c,pk] wvT[c,o]; bias added via the
            # precomputed broadcast tile (also provides the ones column
            # for the softmax denominator). Output is written as fp8
            # k-pair slabs [128, 2, 2, HALF_W] (dim1 = k-tile pair member)
            # so the AV matmuls can run in DoubleRow mode.
            cp, j = divmod(cch, 2)
            if j == 0 and half2 == 0:
                vT[b][cp] = pv.tile(
                    [128, 2, 2, HALF_W], fp8,
                    name=f"vT_{b}_{cp}i{IT[0]}", tag="vT"
                )
            mmv = ps.tile(
                [128, HALF_W], f32, name=f"mmv_{b}_{cch}_{half2}i{IT[0]}", tag="mm"
            )
            for ct in range(NT):
                nc.tensor.matmul(
                    out=mmv[:],
                    lhsT=hn[b][ct][:, 128 * cch : 128 * (cch + 1)],
                    rhs=w_v[:, ct, half2, :],
                    start=(ct == 0),
                    stop=(ct == NT - 1),
                )
            nc.vector.tensor_tensor(
                out=vT[b][cp][:, j, half2, :],
                in0=mmv[:],
                in1=s_bvb[:, half2, :],
                op=ADD,
            )

        # ---------------- attention ----------------
        eT = {}  # (b, h) -> list of 8 tiles
        aout = [[None] * NH for _ in range(B_LOC)]

        def _scores_unit(b, h, cch):
            # eT = exp(scale*s - 2); the constant shift cancels in softmax
            # and keeps exp outputs < 240 (fp8e4 saturates to Inf above).
            cp, j = divmod(cch, 2)
            if cch == 0:
                eT[(b, h)] = [None] * 4
            sps = ps.tile(
                [128, HW], f32, name=f"sps_{b}_{h}_{cch}i{IT[0]}", tag="sc"
            )
            for half in range(2):
                nc.tensor.matmul(
                    out=sps[:, 512 * half : 512 * (half + 1)],
                    lhsT=k_sb[b][h][:, 128 * cch : 128 * (cch + 1)],
                    rhs=q_sb[b][h][:, 512 * half : 512 * (half + 1)],
                    start=True,
                    stop=True,
                )
            if j == 0:
                eT[(b, h)][cp] = pe.tile(
                    [128, 2, HW], fp8,
                    name=f"eT_{b}_{h}_{cp}i{IT[0]}", tag=f"eT{cp}"
                )
            nc.scalar.activation(
                out=eT[(b, h)][cp][:, j, :], in_=sps[:], func=Exp,
                bias=s_m2[:], scale=SCALE
            )

        o2n_cur = {}

        def _av_unit(b, h, m):
            if m == 0:
                # aout stored as [128, 8, 128] m-blocks (contiguous == HW)
                aout[b][h] = pa.tile(
                    [128, 8, 128], bf16, name=f"aout_{b}_{h}i{IT[0]}", tag="aout"
                )
                o2n_cur[(b, h)] = prc.tile(
                    [128, 8, 128], bf16, name=f"o2n_{b}_{h}i{IT[0]}", tag="o2n"
                )
            half2, hh = divmod(h, 2)
            voff = HSTRIDE * hh
            ets = eT[(b, h)]
            o2n = o2n_cur[(b, h)]
            ops = ps.tile(
                [128, HD + 1], f32, name=f"ops_{b}_{h}_{m}i{IT[0]}", tag="av"
            )
            for cp in range(4):
                nc.tensor.matmul(
                    out=ops[:],
                    lhsT=ets[cp][:, :, 128 * m : 128 * (m + 1)],
                    rhs=vT[b][cp][:, :, half2, voff : voff + HD + 1],
                    start=(cp == 0),
                    stop=(cp == 3),
                    perf_mode=DR,
                )
            rcp = prc.tile([128, 1], f32, name=f"rcp_{b}_{h}_{m}i{IT[0]}", tag="rcp")
            nc.vector.reciprocal(out=rcp[:], in_=ops[:, HD : HD + 1])
            nc.vector.tensor_scalar_mul(
                out=o2n[:, m, :], in0=ops[:, 0:HD], scalar1=rcp[:]
            )
            if m == 7:
                # one batched xbar transpose per head: aout[p, m, c] =
                # o2n[c, m, p] (block-transpose of each 128x128 m-chunk)
                nc.sync.dma_start_transpose(out=aout[b][h][:], in_=o2n[:])

        # ---------------- projection + bias + residual ----------------
        o_cur = {}

        def _proj_unit(b, ot, half):
            if half == 0:
                o_cur[(b, ot)] = po.tile(
                    [128, HW], f32, name=f"o_{b}_{ot}i{IT[0]}", tag="o"
                )
            o_sb = o_cur[(b, ot)]
            mmp = ps.tile(
                [128, 512], f32, name=f"mmp_{b}_{ot}_{half}i{IT[0]}", tag="mm"
            )
            for ct in range(NT):
                nc.tensor.matmul(
                    out=mmp[:],
                    lhsT=w_p[:, ct, 128 * ot : 128 * (ot + 1)],
                    rhs=aout[b][ct][:, 4 * half : 4 * (half + 1), :],
                    start=(ct == 0),
                    stop=(ct == NT - 1),
                )
            nc.vector.scalar_tensor_tensor(
                out=o_sb[:, 512 * half : 512 * (half + 1)],
                in0=mmp[:],
                scalar=s_bp[:, ot : ot + 1],
                in1=xt[b][ot][:, 512 * half : 512 * (half + 1)],
                op0=ADD,
                op1=ADD,
            )
            if half == 1:
                nc.gpsimd.dma_start(
                    out=outp[b, 128 * ot : 128 * (ot + 1), :], in_=o_sb[:]
                )

        # ---------------- emission order: software pipeline ----------------
        # Unit-granular interleave: the exp stream (ACT) is paced by one
        # scores-unit per slot; each slot also carries AV units of an
        # earlier head plus GEMM "filler" units so the PE queue head is
        # never blocked on a single dependency chain.
        from collections import deque

        for _it in range(UNROLL):
            IT[0] = _it
            _gn(0)
            _gn(1)

            fill = deque()
            for b2 in range(B_LOC):
                for ot in range(NT):
                    for which in ("q", "k"):
                        for half in range(2):
                            fill.append((_qk_unit, (b2, ot, which, half)))
                if b2 == 0:
                    for cch in range(8):
                        for half2 in range(2):
                            fill.append((_v_unit, (0, cch, half2)))
            for cch in range(8):
                for half2 in range(2):
                    fill.append((_v_unit, (1, cch, half2)))

            def _fill(n):
                for _ in range(n):
                    if fill:
                        f, a = fill.popleft()
                        f(*a)

            # prologue: hn(0) -> q/k for head 0
            _fill(4)
            heads = [(0, 0), (0, 1), (0, 2), (0, 3), (1, 0), (1, 1), (1, 2), (1, 3)]
            # av streams emitted during each head slot (lagging so vT /
            # eT producers are comfortably ahead)
            av_sched = {2: [(0, 0), (0, 1)], 3: [(0, 2)], 4: [(0, 3)],
                        6: [(1, 0), (1, 1)], 7: [(1, 2)]}
            for hi, (b2, h) in enumerate(heads):
                avs = av_sched.get(hi, [])
                for cch in range(8):
                    _scores_unit(b2, h, cch)
                    for abh in avs:
                        _av_unit(abh[0], abh[1], cch)
                    _fill(2 if hi < 2 else 1)
                if hi == 4:
                    for ot in range(NT):
                        for half in range(2):
                            fill.append((_proj_unit, (0, ot, half)))
            # tail: last head's AV, leftover fillers, final projection
            for m in range(8):
                _av_unit(1, 3, m)
                _fill(1)
            _fill(64)
            for ot in range(NT):
                for half in range(2):
                    _proj_unit(1, ot, half)

    nc.compile()
    return nc


def _get_nc():
    if "nc" not in _CACHE:
        _CACHE["nc"] = _build_nc()
    return _CACHE["nc"]


def _prep_shared(wq, bq, wk, bk, wv, bv, wp, bp, gamma, beta):
    bf16 = ml_dtypes.bfloat16
    wqT = np.ascontiguousarray(wq.T).astype(bf16)
    wkT = np.ascontiguousarray(wk.T).astype(bf16)
    wpT = np.ascontiguousarray(wp.T).astype(bf16)
    wvTf = np.ascontiguousarray(wv.T)  # [c, o]
    wvT = np.zeros((C, 2, HALF_W), dtype=bf16)
    bvb = np.zeros((1, 2, HALF_W), dtype=np.float32)
    for h in range(NH):
        half2, hh = divmod(h, 2)
        off = HSTRIDE * hh
        wvT[:, half2, off : off + HD] = wvTf[:, HD * h : HD * (h + 1)].astype(bf16)
        bvb[0, half2, off : off + HD] = bv[HD * h : HD * (h + 1)]
        bvb[0, half2, off + HD] = 1.0  # ones column -> softmax denominator
    bvb = np.broadcast_to(bvb, (128, 2, HALF_W)).astype(bf16)
    as_col = lambda v: np.ascontiguousarray(v.reshape(NT, 128).T).astype(np.float32)
    selS = np.zeros((128, GPT), np.float32)
    for p in range(128):
        selS[p, p // GSZ] = 1.0
    selST = np.ascontiguousarray(selS.T)
    iden = np.eye(128, dtype=bf16)
    return {
        "wqT": wqT,
        "wkT": wkT,
        "wvT": wvT,
        "wpT": wpT,
        "bq2": as_col(bq),
        "bk2": as_col(bk),
        "bvb": np.ascontiguousarray(bvb),
        "bp2": as_col(bp),
        "gam2": as_col(gamma),
        "bet2": as_col(beta),
        "selS": selS,
        "selST": selST,
        "iden": iden,
    }


def kernel(x, gamma, beta, wq, bq, wk, bk, wv, bv, wp, bp, _trace=False):
    from concourse.bass_utils import run_bass_kernel_spmd

    x = np.asarray(x, dtype=np.float32)
    shared = _prep_shared(
        np.asarray(wq, np.float32),
        np.asarray(bq, np.float32),
        np.asarray(wk, np.float32),
        np.asarray(bk, np.float32),
        np.asarray(wv, np.float32),
        np.asarray(bv, np.float32),
        np.asarray(wp, np.float32),
        np.asarray(bp, np.float32),
        np.asarray(gamma, np.float32),
        np.asarray(beta, np.float32),
    )
    xr = x.reshape(B, C, HW).astype(ml_dtypes.bfloat16)
    in_maps = []
    for i in range(NCORES):
        m = dict(shared)
        m["x"] = np.ascontiguousarray(xr[B_LOC * i : B_LOC * (i + 1)])
        in_maps.append(m)

    nc = _get_nc()
    res = run_bass_kernel_spmd(
        nc, in_maps, core_ids=list(range(NCORES)), trace=_trace
    )
    out = np.concatenate([res.results[i]["out"] for i in range(NCORES)], axis=0)
    out = out.reshape(B, C, 32, 32).astype(np.float32)
    if _trace:
        _CACHE["last_exec_time_ns"] = res.exec_time_ns
        _CACHE["last_results"] = res
    return out
